# revision 27
# baseline (speedup 1.0000x reference)
"""GCN encoder (GCNConv + PReLU) distributed Bass kernel for 8 TRN2 NeuronCores.

Reference computation:
    src/dst = edge_index with self loops appended
    deg[v]  = #edges with dst==v (incl. self loop)
    dinv    = rsqrt(deg)
    h       = x @ W
    agg[v]  = sum_{e: dst=v} dinv[src_e]*dinv[v]*h[src_e] + b
    out     = prelu(agg, alpha)

Factored form used on device:
    g[u]    = dinv[u] * h[u]
    out[v]  = prelu(dinv[v] * sum_{e: dst=v} g[src_e] + b)   (self loop = an edge)

Distribution: dst-node ownership sharded over 8 cores. Each core computes its
g shard (xT bf16 is shipped pre-transposed so h = matmul(lhsT=xT, rhs=W)
directly, scaled by rsqrt(deg)); four chunked AllGathers replicate g (bf16)
quarter-shard by quarter-shard so phase 3 can start after the first one; then
each core gathers the 256B g rows of its edges with dma_gather and
segment-sums them on the TensorEngine: for each 128-edge block,
matmul(lhsT=onehot[128e,128dst], rhs=g_rows[128e,128f]) accumulates into a
PSUM [128dst, 128f] window — output land in natural [node, feat] layout, so
the epilogue is just dinv[dst] (per-partition scale), bias + PReLU
(free-dim-broadcast b/alpha tiles), and a direct bf16 row write (host casts
back to f32).

Host side does only integer index work: self loops, bincount, edge binning
into (dst-tile window, src-quarter chunk) groups padded to 128-edge blocks
with per-group block counts (max over cores, so the program stays SPMD), and
int16 gather-index tables (dma_gather needs idx<32768: each AllGather stripe
holds 8 quarter-shards = at most 31744 rows). Uploads are minimized: the idx
table ships once ([16, n/16]) and is replicated to the 8 Q7 core groups on
device; dstrel ships as int8.
"""

import math
import os

import numpy as np
import ml_dtypes

import concourse.bass as bass
import concourse.tile as tile
import concourse.bacc as bacc
from concourse import mybir
from concourse.bass_utils import run_bass_kernel_spmd

N_CORES = 8
P = 128          # partitions / feature dim
WPC = int(os.environ.get("GCN_WPC", "8"))  # windows per dma_gather call group
MAX_CHUNK = 32767  # dma_gather int16 idx limit

BF16 = mybir.dt.bfloat16
F32 = mybir.dt.float32
I16 = mybir.dt.int16
I8 = mybir.dt.int8


def _wrap_idx16(flat):
    """[n] int -> [16, n//16] int16: idx i at partition i%16 position i//16."""
    n = flat.shape[0]
    return np.ascontiguousarray(flat.reshape(n // 16, 16).T.astype(np.int16))


def _quarters(npc_pad):
    """Split a core's padded shard into 4 tile-aligned quarters, each with
    8*rows <= MAX_CHUNK so stripe-local indices fit in int16."""
    jt = npc_pad // P
    per = MAX_CHUNK // (8 * P)          # tiles per quarter (31 for 12544)
    qt = []
    t0 = 0
    while t0 < jt:
        qt.append(min(per, jt - t0))
        t0 += per
    qstart = np.cumsum([0] + qt[:-1]) * P
    qrows = np.asarray(qt) * P
    return qstart, qrows


def _host_prep(x, edge_index, W, b, alpha):
    N, D = x.shape
    assert D == P and N % N_CORES == 0
    npc = N // N_CORES
    npc_pad = ((npc + P - 1) // P) * P
    jt = npc_pad // P
    nwin = jt
    qstart, qrows = _quarters(npc_pad)
    nch = len(qrows)
    assert (8 * qrows <= MAX_CHUNK + 1).all() and 8 * qrows.max() <= MAX_CHUNK

    src = np.concatenate([np.asarray(edge_index[0]), np.arange(N, dtype=np.int64)])
    dst = np.concatenate([np.asarray(edge_index[1]), np.arange(N, dtype=np.int64)])
    deg = np.bincount(dst, minlength=N).astype(np.float32)

    core = dst // npc
    dloc = dst - core * npc
    w = dloc // P
    dst_rel = dloc % P
    cs = src // npc
    r = src % npc
    k = np.searchsorted(qstart, r, side="right") - 1       # src quarter/chunk
    stripe_row = cs * qrows[k] + (r - qstart[k])           # idx within stripe k

    # group id per edge: (core, w, k)
    n_gper = nwin * nch
    gid = (core * nwin + w) * nch + k
    counts = np.bincount(gid, minlength=N_CORES * n_gper)
    cpg = counts.reshape(N_CORES, nwin, nch)
    nb = np.ceil(cpg.max(axis=0) / P).astype(np.int64)     # [nwin, nch]

    # ---- block layout (identical across cores; data differs) ----
    quads = [list(range(q, min(q + WPC, nwin))) for q in range(0, nwin, WPC)]
    blk_base = np.zeros((nwin, nch), np.int64)
    calls = []          # (k, windows, blk_start, nblk)
    nxt = 0
    for wq in quads:
        for kk in range(nch):
            start = nxt
            for ww in wq:
                blk_base[ww, kk] = nxt
                nxt += nb[ww, kk]
            calls.append((kk, wq, start, nxt - start))
    totblk = nxt
    slots = totblk * P

    # ---- slot assignment ----
    order = np.argsort(gid, kind="stable")
    gid_s = gid[order]
    starts = np.zeros(N_CORES * n_gper + 1, np.int64)
    starts[1:] = np.cumsum(counts)
    pos = np.arange(len(order), dtype=np.int64) - starts[gid_s]

    core_s = gid_s // n_gper
    rem = gid_s - core_s * n_gper
    w_s = rem // nch
    k_s = rem - w_s * nch
    assert (pos < nb[w_s, k_s] * P).all()
    slot = blk_base[w_s, k_s] * P + pos

    idx16 = np.zeros((N_CORES, slots), np.int16)
    dstrel = np.full((N_CORES, slots), -1, np.int8)
    idx16[core_s, slot] = stripe_row[order].astype(np.int16)
    dstrel[core_s, slot] = dst_rel[order].astype(np.int8)

    idx_dev = np.stack([_wrap_idx16(idx16[c]) for c in range(N_CORES)])
    dst_dev = np.ascontiguousarray(
        dstrel.reshape(N_CORES, totblk, P).transpose(0, 2, 1))
    iota = np.ascontiguousarray(
        np.broadcast_to(np.arange(P, dtype=np.float32), (P, P))).astype(
            ml_dtypes.bfloat16)

    in_maps = []
    for c in range(N_CORES):
        deg_c = np.ones(npc_pad, np.float32)
        deg_c[:npc] = deg[c * npc:(c + 1) * npc]
        deg_wrap = np.ascontiguousarray(deg_c.reshape(jt, P).T)
        xT_c = np.zeros((P, npc_pad), np.float32)
        xT_c[:, :npc] = np.asarray(x[c * npc:(c + 1) * npc]).T
        in_maps.append({
            "xT": xT_c.astype(ml_dtypes.bfloat16),
            "w": np.ascontiguousarray(W, dtype=np.float32),
            "bias": np.asarray(b, np.float32).reshape(P),
            "alpha": np.asarray(alpha, np.float32).reshape(P),
            "deg_wrap": deg_wrap,
            "idx16": idx_dev[c],
            "dstrel": dst_dev[c],
            "iota": iota,
        })

    meta = dict(npc=npc, npc_pad=npc_pad, nwin=nwin, jt=jt, nch=nch,
                qstart=qstart, qrows=qrows, nb=nb, totblk=totblk,
                calls=calls, blk_base=blk_base)
    return in_maps, meta


def _build_program(meta):
    npc_pad = meta["npc_pad"]
    nwin = meta["nwin"]
    jt = meta["jt"]
    nch = meta["nch"]
    qstart = meta["qstart"]
    qrows = meta["qrows"]
    nb = meta["nb"]
    totblk = meta["totblk"]
    calls = meta["calls"]
    blk_base = meta["blk_base"]
    Act = mybir.ActivationFunctionType

    # last (kk, i) per window that has a matmul, for the PSUM stop flag
    last_ki = {}
    for ww in range(nwin):
        for kk in range(nch):
            if nb[ww, kk] > 0:
                last_ki[ww] = (kk, nb[ww, kk] - 1)

    _nq = int(os.environ.get("GCN_NQ", "4"))
    nc = bacc.Bacc("TRN2", target_bir_lowering=False, debug=False,
                   num_devices=N_CORES, num_swdge_queues=_nq)

    xT_d = nc.dram_tensor("xT", [P, npc_pad], BF16, kind="ExternalInput").ap()
    w_d = nc.dram_tensor("w", [P, P], F32, kind="ExternalInput").ap()
    b_d = nc.dram_tensor("bias", [P], F32, kind="ExternalInput").ap()
    alpha_d = nc.dram_tensor("alpha", [P], F32, kind="ExternalInput").ap()
    degw_d = nc.dram_tensor("deg_wrap", [P, jt], F32, kind="ExternalInput").ap()
    idx_d = nc.dram_tensor("idx16", [16, totblk * P // 16], I16,
                           kind="ExternalInput").ap()
    dstr_d = nc.dram_tensor("dstrel", [P, totblk], I8, kind="ExternalInput").ap()
    iota_d = nc.dram_tensor("iota", [P, P], BF16, kind="ExternalInput").ap()
    out_d = nc.dram_tensor("out", [npc_pad, P], BF16, kind="ExternalOutput").ap()

    with tile.TileContext(nc) as tc:
        with tile.ExitStack() as top:
            cpool = top.enter_context(tc.tile_pool(name="const", bufs=1))
            dpool = top.enter_context(tc.tile_pool(name="dram", bufs=1, space="DRAM"))

            # ---- constants ----
            w_f32 = cpool.tile([P, P], F32, name="w_f32")
            nc.sync.dma_start(out=w_f32[:], in_=w_d[:])
            w_bf = cpool.tile([P, P], BF16, name="w_bf")
            nc.vector.tensor_copy(out=w_bf[:], in_=w_f32[:])

            # per-feature bias/alpha broadcast across partitions (free-dim use)
            b_bc = cpool.tile([P, P], F32, name="b_bc")
            nc.sync.dma_start(
                out=b_bc[:], in_=b_d[:].unsqueeze(0).partition_broadcast(P))
            alpha_bc = cpool.tile([P, P], F32, name="alpha_bc")
            nc.sync.dma_start(
                out=alpha_bc[:],
                in_=alpha_d[:].unsqueeze(0).partition_broadcast(P))

            iota_sb = cpool.tile([P, P], BF16, name="iota_sb")
            nc.sync.dma_start(out=iota_sb[:], in_=iota_d[:])

            # dinv wrapped: partition p, col t -> rsqrt(deg[t*128+p])
            degw_sb = cpool.tile([P, jt], F32, name="degw_sb")
            nc.sync.dma_start(out=degw_sb[:], in_=degw_d[:])
            dinvw_sb = cpool.tile([P, jt], F32, name="dinvw_sb")
            nc.vector.reciprocal(dinvw_sb[:], degw_sb[:])
            nc.scalar.sqrt(dinvw_sb[:], dinvw_sb[:])

            # edge tables: idx replicated on device to the 8 Q7 core groups
            idx_sb = cpool.tile([P, totblk * P // 16], I16, name="idx_sb")
            for gq in range(8):
                nc.sync.dma_start(out=idx_sb[16 * gq:16 * (gq + 1), :], in_=idx_d[:])
            dstrel8_sb = cpool.tile([P, totblk], I8, name="dstrel8_sb")
            nc.sync.dma_start(out=dstrel8_sb[:], in_=dstr_d[:])
            dstrel_sb = cpool.tile([P, totblk], BF16, name="dstrel_sb")
            nc.vector.tensor_copy(out=dstrel_sb[:], in_=dstrel8_sb[:])

            # quarter-shard g tensors and their AllGather stripes
            g_q = [dpool.tile([int(qrows[q]), P], BF16, name=f"g_q{q}")
                   for q in range(nch)]
            g_str = [dpool.tile([N_CORES * int(qrows[q]), P], BF16,
                                addr_space="Shared", name=f"g_str{q}")
                     for q in range(nch)]

            # ---- phase 1: g = dinv * (x @ W), written quarter by quarter ----
            qt0 = (qstart // P).tolist() + [jt]
            with tile.ExitStack() as ph1:
                psH_pool = ph1.enter_context(
                    tc.tile_pool(name="ph1psH", bufs=4, space="PSUM"))
                ph1_pool = ph1.enter_context(tc.tile_pool(name="ph1sb", bufs=4))
                xT_all = ph1_pool.tile([P, npc_pad], BF16, name="xT_all", bufs=1)
                nc.sync.dma_start(out=xT_all[:], in_=xT_d[:])
                for t in range(jt):
                    q = np.searchsorted(qstart, t * P, side="right") - 1
                    h_ps = psH_pool.tile([P, P], F32, tag="h", name="h_ps")
                    nc.tensor.matmul(out=h_ps[:], lhsT=xT_all[:, t * P:(t + 1) * P],
                                     rhs=w_bf[:], start=True, stop=True)
                    g_sb = ph1_pool.tile([P, P], BF16, tag="g", name="g_sb")
                    nc.scalar.activation(out=g_sb[:], in_=h_ps[:], func=Act.Copy,
                                         scale=dinvw_sb[:, t:t + 1])
                    r0 = t * P - int(qstart[q])
                    nc.sync.dma_start(out=g_q[q][r0:r0 + P, :], in_=g_sb[:])

            # ---- phase 2: replicate g, one chunk at a time (overlaps ph3) ----
            _dbg = os.environ.get("GCN_DEBUG", "")
            if "nocc" not in _dbg:
                for q in range(nch):
                    for _ in range(int(os.environ.get("GCN_REP_CC", "1"))):
                        nc.gpsimd.collective_compute(
                            "AllGather",
                            mybir.AluOpType.bypass,
                            replica_groups=[list(range(N_CORES))],
                            ins=[g_q[q][:].opt()],
                            outs=[g_str[q][:].opt()],
                        )

            # ---- phase 3 ----
            if "noph3" in _dbg or "noepi" in _dbg:
                dummy = cpool.tile([P, P], BF16, name="dummy")
                nc.vector.memset(dummy[:], 0.0)
                nc.sync.dma_start(out=out_d[0:P, :], in_=dummy[:])
                if "noph3" in _dbg:
                    calls = []
            with tile.ExitStack() as ph3:
                gat_pool = ph3.enter_context(tc.tile_pool(name="gat", bufs=int(os.environ.get("GCN_GATBUFS", "4"))))
                oh_pool = ph3.enter_context(tc.tile_pool(name="oh", bufs=int(os.environ.get("GCN_OHBUFS", "3"))))
                psw_pool = ph3.enter_context(
                    tc.tile_pool(name="psw", bufs=8, space="PSUM"))
                epi_pool = ph3.enter_context(tc.tile_pool(name="epi", bufs=3))

                _rep_gat = int(os.environ.get("GCN_REP_GATHER", "1"))
                _rep_oh = int(os.environ.get("GCN_REP_OH", "1"))
                _rep_mm = int(os.environ.get("GCN_REP_MM", "1"))
                psw = {}     # window -> (bank tile, col offset); 4 windows/bank
                for ci, (kk, wq, blk_start, nblk) in enumerate(calls):
                    if kk == 0:
                        banks = [psw_pool.tile([P, 4 * P], F32, tag="psw",
                                               name="psw")
                                 for _ in range((len(wq) + 3) // 4)]
                        for bt in banks:
                            nc.vector.memset(bt[:], 0.0)
                        for j, ww in enumerate(wq):
                            psw[ww] = (banks[j // 4], (j % 4) * P)
                    if nblk > 0:
                        for _r in range(_rep_gat):
                            gat = gat_pool.tile([P, nblk, P], BF16, tag="gat",
                                                name="gat")
                            if "nogather" in _dbg:
                                nc.vector.memset(gat[:], 0.0)
                            else:
                                nc.gpsimd.dma_gather(
                                    out_ap=gat[:],
                                    in_ap=g_str[kk][:],
                                    idxs_ap=idx_sb[:, blk_start * P // 16:
                                                   (blk_start + nblk) * P // 16],
                                    num_idxs=nblk * P,
                                    num_idxs_reg=nblk * P,
                                    elem_size=P,
                                    single_packet=False,
                                    queue_num=ci % _nq,
                                )
                        for _r in range(_rep_oh):
                            oh = oh_pool.tile([P, nblk, P], BF16, tag="oh",
                                              name="oh")
                            if "noonehot" in _dbg:
                                nc.vector.memset(oh[:], 0.0)
                            else:
                                nc.vector.tensor_tensor(
                                    out=oh[:],
                                    in0=dstrel_sb[:, blk_start:blk_start + nblk]
                                        .unsqueeze(2).to_broadcast([P, nblk, P]),
                                    in1=iota_sb[:].unsqueeze(1)
                                        .to_broadcast([P, nblk, P]),
                                    op=mybir.AluOpType.is_equal,
                                )
                    for ww in wq:
                        bt, c0 = psw[ww]
                        for i in range(nb[ww, kk]):
                            if "nomm" in _dbg:
                                continue
                            blk = blk_base[ww, kk] - blk_start + i
                            is_last = (last_ki.get(ww) == (kk, i))
                            for _r in range(_rep_mm):
                                nc.tensor.matmul(
                                    out=bt[:, c0:c0 + P],
                                    lhsT=oh[:, blk, :],
                                    rhs=gat[:, blk, :],
                                    start=False,
                                    stop=(is_last and _r == _rep_mm - 1),
                                    skip_group_check=True,
                                )
                        if kk < nch - 1:
                            continue
                        psw.pop(ww)
                        if "noepi" in _dbg:
                            continue
                        # ---- epilogue for window ww ----
                        # z = prelu(dinv[dst]*agg + b): dinv is per-partition,
                        # b/alpha broadcast along partitions (per-feature).
                        s = epi_pool.tile([P, P], F32, tag="s", name="s")
                        nc.scalar.activation(out=s[:], in_=bt[:, c0:c0 + P],
                                             func=Act.Copy,
                                             scale=dinvw_sb[:, ww:ww + 1])
                        ub = epi_pool.tile([P, P], F32, tag="ub", name="ub")
                        nc.vector.tensor_add(ub[:], s[:], b_bc[:])
                        rp = epi_pool.tile([P, P], F32, tag="rp", name="rp")
                        nc.scalar.activation(out=rp[:], in_=ub[:], func=Act.Relu)
                        qn = epi_pool.tile([P, P], F32, tag="qn", name="qn")
                        nc.scalar.activation(out=qn[:], in_=ub[:], func=Act.Relu,
                                             scale=-1.0)
                        aq = epi_pool.tile([P, P], F32, tag="aq", name="aq")
                        nc.vector.tensor_mul(aq[:], qn[:], alpha_bc[:])
                        z = epi_pool.tile([P, P], BF16, tag="z", name="z")
                        nc.vector.tensor_sub(z[:], rp[:], aq[:])
                        row0 = ww * P
                        nc.sync.dma_start(out=out_d[row0:row0 + P, :], in_=z[:])

    nc.compile()
    return nc


_CACHE = {}


def kernel(x, edge_index, W, b, alpha):
    x = np.asarray(x)
    edge_index = np.asarray(edge_index)

    in_maps, meta = _host_prep(x, edge_index, np.asarray(W), np.asarray(b),
                               np.asarray(alpha))
    key = (x.shape, edge_index.shape, meta["totblk"])
    if key not in _CACHE:
        _CACHE[key] = _build_program(meta)
    nc = _CACHE[key]

    r = run_bass_kernel_spmd(nc, in_maps, list(range(N_CORES)))
    npc = meta["npc"]
    out = np.concatenate([np.asarray(r.results[c]["out"])[:npc]
                          for c in range(N_CORES)], axis=0)
    return out.astype(np.float32)


# revision 29
# speedup vs baseline: 1.3053x; 1.3053x over previous
"""GCN encoder (GCNConv + PReLU) distributed Bass kernel for 8 TRN2 NeuronCores.

Reference computation:
    src/dst = edge_index with self loops appended
    deg[v]  = #edges with dst==v (incl. self loop)
    dinv    = rsqrt(deg)
    h       = x @ W
    agg[v]  = sum_{e: dst=v} dinv[src_e]*dinv[v]*h[src_e] + b
    out     = prelu(agg, alpha)

Factored form used on device:
    g[u]    = dinv[u] * h[u]
    out[v]  = prelu(dinv[v] * sum_{e: dst=v} g[src_e] + b)   (self loop = an edge)

Distribution: dst-node ownership sharded over 8 cores. Each core computes its
g shard (xT bf16 is shipped pre-transposed so h = matmul(lhsT=xT, rhs=W)
directly, scaled by rsqrt(deg)); four chunked AllGathers replicate g (bf16)
quarter-shard by quarter-shard so phase 3 can start after the first one; then
each core gathers the 256B g rows of its edges with dma_gather and
segment-sums them on the TensorEngine: for each 128-edge block,
matmul(lhsT=onehot[128e,128dst], rhs=g_rows[128e,128f]) accumulates into a
PSUM [128dst, 128f] window — output land in natural [node, feat] layout, so
the epilogue is just dinv[dst] (per-partition scale), bias + PReLU
(free-dim-broadcast b/alpha tiles), and a direct bf16 row write (host casts
back to f32).

Host side does only integer index work: self loops, bincount, edge binning
into (dst-tile window, src-quarter chunk) groups padded to 128-edge blocks
with per-group block counts (max over cores, so the program stays SPMD), and
int16 gather-index tables (dma_gather needs idx<32768: each AllGather stripe
holds 8 quarter-shards = at most 31744 rows). Uploads are minimized: the idx
table ships once ([16, n/16]) and is replicated to the 8 Q7 core groups on
device; dstrel ships as int8.
"""

import math
import os

import numpy as np
import ml_dtypes

import concourse.bass as bass
import concourse.tile as tile
import concourse.bacc as bacc
from concourse import mybir
from concourse.bass_utils import run_bass_kernel_spmd

N_CORES = 8
P = 128          # partitions / feature dim
WPC = int(os.environ.get("GCN_WPC", "8"))  # windows per dma_gather call group
MAX_CHUNK = 32767  # dma_gather int16 idx limit

BF16 = mybir.dt.bfloat16
F32 = mybir.dt.float32
I16 = mybir.dt.int16
I8 = mybir.dt.int8


def _wrap_idx16(flat):
    """[n] int -> [16, n//16] int16: idx i at partition i%16 position i//16."""
    n = flat.shape[0]
    return np.ascontiguousarray(flat.reshape(n // 16, 16).T.astype(np.int16))


def _quarters(npc_pad):
    """Split a core's padded shard into 4 tile-aligned quarters, each with
    8*rows <= MAX_CHUNK so stripe-local indices fit in int16."""
    jt = npc_pad // P
    per = MAX_CHUNK // (8 * P)          # tiles per quarter (31 for 12544)
    qt = []
    t0 = 0
    while t0 < jt:
        qt.append(min(per, jt - t0))
        t0 += per
    qstart = np.cumsum([0] + qt[:-1]) * P
    qrows = np.asarray(qt) * P
    return qstart, qrows


def _host_prep(x, edge_index, W, b, alpha):
    N, D = x.shape
    assert D == P and N % N_CORES == 0
    npc = N // N_CORES
    npc_pad = ((npc + P - 1) // P) * P
    jt = npc_pad // P
    nwin = jt
    qstart, qrows = _quarters(npc_pad)
    nch = len(qrows)
    assert (8 * qrows <= MAX_CHUNK + 1).all() and 8 * qrows.max() <= MAX_CHUNK

    # deg counts self loops (reference semantics); the self-loop message
    # dinv[v]*g[v] is added analytically in the epilogue, so the edge tables
    # hold only the real edges.
    src = np.asarray(edge_index[0])
    dst = np.asarray(edge_index[1])
    deg = (np.bincount(dst, minlength=N) + 1).astype(np.float32)

    core = dst // npc
    dloc = dst - core * npc
    w = dloc // P
    dst_rel = dloc % P
    cs = src // npc
    r = src % npc
    k = np.searchsorted(qstart, r, side="right") - 1       # src quarter/chunk
    stripe_row = cs * qrows[k] + (r - qstart[k])           # idx within stripe k

    # group id per edge: (core, w, k)
    n_gper = nwin * nch
    gid = (core * nwin + w) * nch + k
    counts = np.bincount(gid, minlength=N_CORES * n_gper)
    cpg = counts.reshape(N_CORES, nwin, nch)
    nb = np.ceil(cpg.max(axis=0) / P).astype(np.int64)     # [nwin, nch]

    # ---- block layout (identical across cores; data differs) ----
    quads = [list(range(q, min(q + WPC, nwin))) for q in range(0, nwin, WPC)]
    blk_base = np.zeros((nwin, nch), np.int64)
    calls = []          # (k, windows, blk_start, nblk)
    nxt = 0
    for wq in quads:
        for kk in range(nch):
            start = nxt
            for ww in wq:
                blk_base[ww, kk] = nxt
                nxt += nb[ww, kk]
            calls.append((kk, wq, start, nxt - start))
    totblk = nxt
    slots = totblk * P

    # ---- slot assignment ----
    order = np.argsort(gid, kind="stable")
    gid_s = gid[order]
    starts = np.zeros(N_CORES * n_gper + 1, np.int64)
    starts[1:] = np.cumsum(counts)
    pos = np.arange(len(order), dtype=np.int64) - starts[gid_s]

    core_s = gid_s // n_gper
    rem = gid_s - core_s * n_gper
    w_s = rem // nch
    k_s = rem - w_s * nch
    assert (pos < nb[w_s, k_s] * P).all()
    slot = blk_base[w_s, k_s] * P + pos

    idx16 = np.zeros((N_CORES, slots), np.int16)
    dstrel = np.full((N_CORES, slots), -1, np.int8)
    idx16[core_s, slot] = stripe_row[order].astype(np.int16)
    dstrel[core_s, slot] = dst_rel[order].astype(np.int8)

    idx_dev = np.stack([_wrap_idx16(idx16[c]) for c in range(N_CORES)])
    dst_dev = np.ascontiguousarray(
        dstrel.reshape(N_CORES, totblk, P).transpose(0, 2, 1))
    iota = np.ascontiguousarray(
        np.broadcast_to(np.arange(P, dtype=np.float32), (P, P))).astype(
            ml_dtypes.bfloat16)

    in_maps = []
    for c in range(N_CORES):
        deg_c = np.ones(npc_pad, np.float32)
        deg_c[:npc] = deg[c * npc:(c + 1) * npc]
        deg_wrap = np.ascontiguousarray(deg_c.reshape(jt, P).T)
        xT_c = np.zeros((P, npc_pad), np.float32)
        xT_c[:, :npc] = np.asarray(x[c * npc:(c + 1) * npc]).T
        in_maps.append({
            "xT": xT_c.astype(ml_dtypes.bfloat16),
            "w": np.ascontiguousarray(W, dtype=np.float32),
            "bias": np.asarray(b, np.float32).reshape(P),
            "alpha": np.asarray(alpha, np.float32).reshape(P),
            "deg_wrap": deg_wrap,
            "idx16": idx_dev[c],
            "dstrel": dst_dev[c],
            "iota": iota,
        })

    meta = dict(npc=npc, npc_pad=npc_pad, nwin=nwin, jt=jt, nch=nch,
                qstart=qstart, qrows=qrows, nb=nb, totblk=totblk,
                calls=calls, blk_base=blk_base)
    return in_maps, meta


def _build_program(meta):
    npc_pad = meta["npc_pad"]
    nwin = meta["nwin"]
    jt = meta["jt"]
    nch = meta["nch"]
    qstart = meta["qstart"]
    qrows = meta["qrows"]
    nb = meta["nb"]
    totblk = meta["totblk"]
    calls = meta["calls"]
    blk_base = meta["blk_base"]
    Act = mybir.ActivationFunctionType

    # last (kk, i) per window that has a matmul, for the PSUM stop flag
    last_ki = {}
    for ww in range(nwin):
        for kk in range(nch):
            if nb[ww, kk] > 0:
                last_ki[ww] = (kk, nb[ww, kk] - 1)

    _nq = int(os.environ.get("GCN_NQ", "4"))
    nc = bacc.Bacc("TRN2", target_bir_lowering=False, debug=False,
                   num_devices=N_CORES, num_swdge_queues=_nq)

    xT_d = nc.dram_tensor("xT", [P, npc_pad], BF16, kind="ExternalInput").ap()
    w_d = nc.dram_tensor("w", [P, P], F32, kind="ExternalInput").ap()
    b_d = nc.dram_tensor("bias", [P], F32, kind="ExternalInput").ap()
    alpha_d = nc.dram_tensor("alpha", [P], F32, kind="ExternalInput").ap()
    degw_d = nc.dram_tensor("deg_wrap", [P, jt], F32, kind="ExternalInput").ap()
    idx_d = nc.dram_tensor("idx16", [16, totblk * P // 16], I16,
                           kind="ExternalInput").ap()
    dstr_d = nc.dram_tensor("dstrel", [P, totblk], I8, kind="ExternalInput").ap()
    iota_d = nc.dram_tensor("iota", [P, P], BF16, kind="ExternalInput").ap()
    out_d = nc.dram_tensor("out", [npc_pad, P], BF16, kind="ExternalOutput").ap()

    with tile.TileContext(nc) as tc:
        with tile.ExitStack() as top:
            cpool = top.enter_context(tc.tile_pool(name="const", bufs=1))
            dpool = top.enter_context(tc.tile_pool(name="dram", bufs=1, space="DRAM"))

            # ---- constants ----
            w_f32 = cpool.tile([P, P], F32, name="w_f32")
            nc.sync.dma_start(out=w_f32[:], in_=w_d[:])
            w_bf = cpool.tile([P, P], BF16, name="w_bf")
            nc.vector.tensor_copy(out=w_bf[:], in_=w_f32[:])

            # per-feature bias/alpha broadcast across partitions (free-dim use)
            b_bc = cpool.tile([P, P], F32, name="b_bc")
            nc.sync.dma_start(
                out=b_bc[:], in_=b_d[:].unsqueeze(0).partition_broadcast(P))
            alpha_bc = cpool.tile([P, P], F32, name="alpha_bc")
            nc.sync.dma_start(
                out=alpha_bc[:],
                in_=alpha_d[:].unsqueeze(0).partition_broadcast(P))

            iota_sb = cpool.tile([P, P], BF16, name="iota_sb")
            nc.sync.dma_start(out=iota_sb[:], in_=iota_d[:])

            # dinv wrapped: partition p, col t -> rsqrt(deg[t*128+p])
            degw_sb = cpool.tile([P, jt], F32, name="degw_sb")
            nc.sync.dma_start(out=degw_sb[:], in_=degw_d[:])
            dinvw_sb = cpool.tile([P, jt], F32, name="dinvw_sb")
            nc.vector.reciprocal(dinvw_sb[:], degw_sb[:])
            nc.scalar.sqrt(dinvw_sb[:], dinvw_sb[:])

            # edge tables: idx replicated on device to the 8 Q7 core groups
            idx_sb = cpool.tile([P, totblk * P // 16], I16, name="idx_sb")
            for gq in range(8):
                nc.sync.dma_start(out=idx_sb[16 * gq:16 * (gq + 1), :], in_=idx_d[:])
            dstrel8_sb = cpool.tile([P, totblk], I8, name="dstrel8_sb")
            nc.sync.dma_start(out=dstrel8_sb[:], in_=dstr_d[:])
            dstrel_sb = cpool.tile([P, totblk], BF16, name="dstrel_sb")
            nc.vector.tensor_copy(out=dstrel_sb[:], in_=dstrel8_sb[:])

            # quarter-shard g tensors and their AllGather stripes
            g_q = [dpool.tile([int(qrows[q]), P], BF16, name=f"g_q{q}")
                   for q in range(nch)]
            g_str = [dpool.tile([N_CORES * int(qrows[q]), P], BF16,
                                addr_space="Shared", name=f"g_str{q}")
                     for q in range(nch)]

            # ---- phase 1: g = dinv * (x @ W), written quarter by quarter ----
            qt0 = (qstart // P).tolist() + [jt]
            with tile.ExitStack() as ph1:
                psH_pool = ph1.enter_context(
                    tc.tile_pool(name="ph1psH", bufs=4, space="PSUM"))
                ph1_pool = ph1.enter_context(tc.tile_pool(name="ph1sb", bufs=4))
                xT_all = ph1_pool.tile([P, npc_pad], BF16, name="xT_all", bufs=1)
                nc.sync.dma_start(out=xT_all[:], in_=xT_d[:])
                for t in range(jt):
                    q = np.searchsorted(qstart, t * P, side="right") - 1
                    h_ps = psH_pool.tile([P, P], F32, tag="h", name="h_ps")
                    nc.tensor.matmul(out=h_ps[:], lhsT=xT_all[:, t * P:(t + 1) * P],
                                     rhs=w_bf[:], start=True, stop=True)
                    g_sb = ph1_pool.tile([P, P], BF16, tag="g", name="g_sb")
                    nc.scalar.activation(out=g_sb[:], in_=h_ps[:], func=Act.Copy,
                                         scale=dinvw_sb[:, t:t + 1])
                    r0 = t * P - int(qstart[q])
                    nc.sync.dma_start(out=g_q[q][r0:r0 + P, :], in_=g_sb[:])

            # ---- phase 2: replicate g, one chunk at a time (overlaps ph3) ----
            _dbg = os.environ.get("GCN_DEBUG", "")
            if "nocc" not in _dbg:
                for q in range(nch):
                    for _ in range(int(os.environ.get("GCN_REP_CC", "1"))):
                        nc.gpsimd.collective_compute(
                            "AllGather",
                            mybir.AluOpType.bypass,
                            replica_groups=[list(range(N_CORES))],
                            ins=[g_q[q][:].opt()],
                            outs=[g_str[q][:].opt()],
                        )

            # ---- phase 3 ----
            if "noph3" in _dbg or "noepi" in _dbg:
                dummy = cpool.tile([P, P], BF16, name="dummy")
                nc.vector.memset(dummy[:], 0.0)
                nc.sync.dma_start(out=out_d[0:P, :], in_=dummy[:])
                if "noph3" in _dbg:
                    calls = []
            with tile.ExitStack() as ph3:
                gat_pool = ph3.enter_context(tc.tile_pool(name="gat", bufs=int(os.environ.get("GCN_GATBUFS", "4"))))
                oh_pool = ph3.enter_context(tc.tile_pool(name="oh", bufs=int(os.environ.get("GCN_OHBUFS", "3"))))
                psw_pool = ph3.enter_context(
                    tc.tile_pool(name="psw", bufs=8, space="PSUM"))
                epi_pool = ph3.enter_context(tc.tile_pool(name="epi", bufs=3))

                _rep_gat = int(os.environ.get("GCN_REP_GATHER", "1"))
                _rep_oh = int(os.environ.get("GCN_REP_OH", "1"))
                _rep_mm = int(os.environ.get("GCN_REP_MM", "1"))
                psw = {}     # window -> (bank tile, col offset); 4 windows/bank
                for ci, (kk, wq, blk_start, nblk) in enumerate(calls):
                    if kk == 0:
                        banks = [psw_pool.tile([P, 4 * P], F32, tag="psw",
                                               name="psw")
                                 for _ in range((len(wq) + 3) // 4)]
                        for bt in banks:
                            nc.vector.memset(bt[:], 0.0)
                        for j, ww in enumerate(wq):
                            psw[ww] = (banks[j // 4], (j % 4) * P)
                    if nblk > 0:
                        for _r in range(_rep_gat):
                            gat = gat_pool.tile([P, nblk, P], BF16, tag="gat",
                                                name="gat")
                            if "nogather" in _dbg:
                                nc.vector.memset(gat[:], 0.0)
                            else:
                                nc.gpsimd.dma_gather(
                                    out_ap=gat[:],
                                    in_ap=g_str[kk][:],
                                    idxs_ap=idx_sb[:, blk_start * P // 16:
                                                   (blk_start + nblk) * P // 16],
                                    num_idxs=nblk * P,
                                    num_idxs_reg=nblk * P,
                                    elem_size=P,
                                    single_packet=False,
                                    queue_num=ci % _nq,
                                )
                        for _r in range(_rep_oh):
                            oh = oh_pool.tile([P, nblk, P], BF16, tag="oh",
                                              name="oh")
                            if "noonehot" in _dbg:
                                nc.vector.memset(oh[:], 0.0)
                            else:
                                nc.vector.tensor_tensor(
                                    out=oh[:],
                                    in0=dstrel_sb[:, blk_start:blk_start + nblk]
                                        .unsqueeze(2).to_broadcast([P, nblk, P]),
                                    in1=iota_sb[:].unsqueeze(1)
                                        .to_broadcast([P, nblk, P]),
                                    op=mybir.AluOpType.is_equal,
                                )
                    # interleave matmuls across windows so consecutive PE ops
                    # hit different PSUM banks and pipeline
                    if "nomm" not in _dbg:
                        mx = max((int(nb[ww, kk]) for ww in wq), default=0)
                        for i in range(mx):
                            for ww in wq:
                                if i >= nb[ww, kk]:
                                    continue
                                bt, c0 = psw[ww]
                                blk = blk_base[ww, kk] - blk_start + i
                                is_last = (last_ki.get(ww) == (kk, i))
                                for _r in range(_rep_mm):
                                    nc.tensor.matmul(
                                        out=bt[:, c0:c0 + P],
                                        lhsT=oh[:, blk, :],
                                        rhs=gat[:, blk, :],
                                        start=False,
                                        stop=(is_last and _r == _rep_mm - 1),
                                        skip_group_check=True,
                                    )
                    if kk < nch - 1:
                        continue
                    for ww in wq:
                        bt, c0 = psw.pop(ww)
                        if "noepi" in _dbg:
                            continue
                        # ---- epilogue for window ww ----
                        # agg += self-loop message g[v]; then
                        # z = prelu(dinv[dst]*agg + b): dinv is per-partition,
                        # b/alpha broadcast along partitions (per-feature).
                        q = int(np.searchsorted(qstart, ww * P, side="right")) - 1
                        r0 = ww * P - int(qstart[q])
                        gown = epi_pool.tile([P, P], BF16, tag="go", name="gown")
                        nc.sync.dma_start(out=gown[:], in_=g_q[q][r0:r0 + P, :])
                        u = epi_pool.tile([P, P], F32, tag="u", name="u")
                        nc.vector.tensor_add(u[:], bt[:, c0:c0 + P], gown[:])
                        s = epi_pool.tile([P, P], F32, tag="s", name="s")
                        nc.scalar.activation(out=s[:], in_=u[:],
                                             func=Act.Copy,
                                             scale=dinvw_sb[:, ww:ww + 1])
                        ub = epi_pool.tile([P, P], F32, tag="ub", name="ub")
                        nc.vector.tensor_add(ub[:], s[:], b_bc[:])
                        rp = epi_pool.tile([P, P], F32, tag="rp", name="rp")
                        nc.scalar.activation(out=rp[:], in_=ub[:], func=Act.Relu)
                        qn = epi_pool.tile([P, P], F32, tag="qn", name="qn")
                        nc.scalar.activation(out=qn[:], in_=ub[:], func=Act.Relu,
                                             scale=-1.0)
                        aq = epi_pool.tile([P, P], F32, tag="aq", name="aq")
                        nc.vector.tensor_mul(aq[:], qn[:], alpha_bc[:])
                        z = epi_pool.tile([P, P], BF16, tag="z", name="z")
                        nc.vector.tensor_sub(z[:], rp[:], aq[:])
                        row0 = ww * P
                        nc.sync.dma_start(out=out_d[row0:row0 + P, :], in_=z[:])

    nc.compile()
    return nc


_CACHE = {}


def kernel(x, edge_index, W, b, alpha):
    x = np.asarray(x)
    edge_index = np.asarray(edge_index)

    in_maps, meta = _host_prep(x, edge_index, np.asarray(W), np.asarray(b),
                               np.asarray(alpha))
    key = (x.shape, edge_index.shape, meta["totblk"])
    if key not in _CACHE:
        _CACHE[key] = _build_program(meta)
    nc = _CACHE[key]

    r = run_bass_kernel_spmd(nc, in_maps, list(range(N_CORES)))
    npc = meta["npc"]
    out = np.concatenate([np.asarray(r.results[c]["out"])[:npc]
                          for c in range(N_CORES)], axis=0)
    return out.astype(np.float32)
